# revision 16
# baseline (speedup 1.0000x reference)
"""Trainium2 Bass kernel for the max-plus (tropical) forward-backward chain.

Problem: 2-state max-plus message passing over length-4096 chains, batch 2048.
    psi = j * outer([-.5,.5],[-.5,.5]);  phi[b,i,s] = b[obs[b,i]] * values[s]
    forward/backward max-plus message scans + belief assembly.

Algorithm (exact reduction; valid for j > 0 and each b_s either >= 0 or
<= -j, which holds for the staged inputs):
  The message difference d = msg[1]-msg[0] follows a clamped walk
  d' = clamp(d + tau_i, -j/2, +j/2), tau_i = b[obs_i].  With e = -d this is
  e' = max(A(obs), e) + B(obs): one hardware tensor_tensor_scan(max, add)
  per direction.  The message level msg[0] accumulates
  v_i = (a - c_i) + rho_i,  rho_i = relu(-e_i - (j/2 - tau_i)).
  Forward prefix Rf + backward suffix Rb combine into one more scan:
  H_i = (rho_f_{i-1} + H_{i-1}) - rho_b_i, seeded with sum(rho_b) + K1 so the
  scan emits out0 = H + K1 directly.  Then
    out1 = (out0 + b0) + v1,   v1 = beta*o - ef_sh - eb_rev  (beta = b1-b0)
  using per-row scalar K1 = 0.5*sum(j/2 - tau) - j/4 derived from the
  accumulated sum of the A tile.

Sharding: pure data parallel - batch 2048 -> 8 cores x 256 sequences
(2 partition-tiles of 128); the L-scans stay on-device per core.

Engine split per [128, 4096] tile:
  DVE    : 3 scans (ef, eb, H/out0) + 2 stt (xf, xb) + v1 TT + out1 stt
  ScalarE: A build (int8->f32 affine, accum for K1), 2 relus
  GpSimd : B build (int8->f32 affine), pad-column memsets (note:
           gpsimd (add,max) op pair measured pathologically slow - avoid)
  sync   : DMAs

Measured (8 cores, axon trn2): HW exec ~113-117 us per NEFF execution,
rel-l2 error vs reference 2.9e-06.  DVE busy ~96 us of that (3 scans at
2 cyc/elem + 4 pointwise ops at 1 cyc/elem over 2 x [128, 4096] tiles);
~24 us is fixed preamble/drain + startup DMA latency.
"""

import sys

import numpy as np

if "/opt/trn_rl_repo" not in sys.path:
    sys.path.insert(0, "/opt/trn_rl_repo")

B_FULL, L_FULL, N_CORES = 2048, 4096, 8

_PROGRAM_CACHE = {}


def _build_program(j, b0, b1, B_c, L):
    """Build + compile the per-core program: obs int8 [B_c, L] -> out f32 [B_c,2,L]."""
    import concourse.bacc as bacc
    import concourse.mybir as mybir
    import concourse.tile as tile

    f32 = mybir.dt.float32
    Alu = mybir.AluOpType
    Act = mybir.ActivationFunctionType

    a = j / 4.0
    twoa = j / 2.0
    beta = b1 - b0
    taus = [b0, b1]
    As, Bs = [], []
    for t in taus:
        if t >= 0:
            As.append(t - twoa)  # only the upper clamp can bind
            Bs.append(-t)
        else:  # t <= -j: guaranteed reset to the bottom clamp
            As.append(twoa)
            Bs.append(0.0)
    sA, bA = As[1] - As[0], As[0]
    sB, bB = Bs[1] - Bs[0], Bs[0]
    assert abs(sA) > 1e-8  # caller guarantees (K1 derives from sum(A))
    # K1 = 0.5*sum(Omega) - a, Omega = twoa - tau:
    #   sumOm = twoa*L - (b0*L + beta*N1), N1 = (sumA - bA*L)/sA
    # => K1 = cK0 + cK1 * sumA
    cK1 = -0.5 * beta / sA
    cK0 = 0.5 * (twoa * L - b0 * L + beta * bA * L / sA) - a
    bR = -(twoa - b0)  # relu bias: rho = relu(xf + bR), xf = beta*o - e_sh

    n_tiles = B_c // 128
    assert B_c % 128 == 0

    nc = bacc.Bacc("TRN2", target_bir_lowering=False, debug=False)
    CH = 4
    W = L // CH
    # obs arrives block-packed (host layout transform): block (t, c) holds
    # rows [t*128,(t+1)*128) x cols [c*W,(c+1)*W) contiguously, so every
    # chunk DMA is a single contiguous 128KB burst.
    obs_d = nc.dram_tensor(
        "obs", [(B_c // 128) * CH, 128, W], mybir.dt.int8, kind="ExternalInput"
    ).ap()
    out_d = nc.dram_tensor("out", [B_c, 2, L], f32, kind="ExternalOutput").ap()

    with tile.TileContext(nc) as tc:
        with tc.tile_pool(name="const", bufs=1) as cpool, tc.tile_pool(
            name="work", bufs=1
        ) as pool, tc.tile_pool(name="inp", bufs=2) as ipool:
            bA_t = cpool.tile([128, 1], f32, tag="bA")
            bR_t = cpool.tile([128, 1], f32, tag="bR")
            nc.vector.memset(bA_t[:], bA)
            nc.vector.memset(bR_t[:], bR)

            for t in range(n_tiles):
                rows = slice(t * 128, (t + 1) * 128)
                o = ipool.tile([128, L], mybir.dt.int8, tag="o")
                A = pool.tile([128, L], f32, tag="A")
                B = pool.tile([128, L], f32, tag="B")
                sA_p = [pool.tile([128, 1], f32, name=f"sAp{t}_{c}", tag=f"sA{c}") for c in range(CH)]
                efb = pool.tile([128, L + 1], f32, tag="efb")
                ebb = pool.tile([128, L + 1], f32, tag="ebb")
                nc.gpsimd.memset(efb[:, 0:1], 0.0)
                nc.gpsimd.memset(ebb[:, 0:1], 0.0)

                for c in range(CH):
                    cs = slice(c * W, (c + 1) * W)
                    nc.sync.dma_start(out=o[:, cs], in_=obs_d[t * CH + c, :, :])
                    nc.scalar.activation(
                        A[:, cs], o[:, cs], Act.Identity, bias=bA_t[:], scale=sA,
                        accum_out=sA_p[c][:],
                    )
                    nc.gpsimd.tensor_scalar(
                        B[:, cs], o[:, cs], sB, bB, Alu.mult, Alu.add
                    )
                # forward walk, chunked + chained: state = max(A_t, state) + B_t
                for c in range(CH):
                    cs = slice(c * W, (c + 1) * W)
                    nc.vector.tensor_tensor_scan(
                        efb[:, 1 + c * W : 1 + (c + 1) * W], A[:, cs], B[:, cs],
                        0.0 if c == 0 else efb[:, c * W : c * W + 1],
                        Alu.max, Alu.add,
                    )
                # backward walk (single full-length scan on reversed inputs;
                # full-length reversed reads carry no chunk penalty)
                nc.vector.tensor_tensor_scan(
                    ebb[:, 1 : L + 1], A[:, ::-1], B[:, ::-1], 0.0,
                    Alu.max, Alu.add,
                )

                xf = pool.tile([128, L], f32, tag="xf")
                rfb = pool.tile([128, L + 1], f32, tag="rfb")
                xb = pool.tile([128, L], f32, tag="xb")
                rbb = pool.tile([128, L], f32, tag="rbb")
                rb_p = [pool.tile([128, 1], f32, name=f"rbp{t}_{c}", tag=f"rb{c}") for c in range(CH)]
                nc.gpsimd.memset(rfb[:, 0:1], 0.0)
                for c in range(CH):
                    cs = slice(c * W, (c + 1) * W)
                    # xf = beta*o - ef_sh ; rho_f = relu(xf + bR)
                    nc.vector.scalar_tensor_tensor(
                        xf[:, cs], o[:, cs], beta, efb[:, cs], Alu.mult, Alu.subtract
                    )
                    nc.scalar.activation(
                        rfb[:, 1 + c * W : 1 + (c + 1) * W], xf[:, cs], Act.Relu,
                        bias=bR_t[:],
                    )
                    # xb in FORWARD layout: xb_k = beta*o_k - eb_fwd_k, where
                    # eb_fwd_k = ebb[:, L-1-k] (reversed read of the bwd walk)
                    nc.vector.scalar_tensor_tensor(
                        xb[:, cs], o[:, cs], beta,
                        ebb[:, L - (c + 1) * W : L - c * W][:, ::-1],
                        Alu.mult, Alu.subtract,
                    )
                    nc.scalar.activation(
                        rbb[:, cs], xb[:, cs], Act.Relu, bias=bR_t[:],
                        accum_out=rb_p[c][:],
                    )

                # seed = K1 + sum(rho_b),  K1 = cK0 + cK1*sumA  (tiny [128,1]
                # ops on gpsimd, off the DVE critical path)
                sA_t = pool.tile([128, 1], f32, tag="sAt")
                nc.gpsimd.tensor_tensor(sA_t[:], sA_p[0][:], sA_p[1][:], Alu.add)
                for c in range(2, CH):
                    nc.gpsimd.tensor_tensor(sA_t[:], sA_t[:], sA_p[c][:], Alu.add)
                seed = pool.tile([128, 1], f32, tag="seed")
                nc.gpsimd.tensor_scalar(seed[:], sA_t[:], cK1, cK0, Alu.mult, Alu.add)
                for c in range(CH):
                    nc.gpsimd.tensor_tensor(seed[:], seed[:], rb_p[c][:], Alu.add)

                # v1 = xb_fwd - ef_sh (independent of H; emitted first so the
                # DVE can fill the rb-accum wait)
                out0 = pool.tile([128, L], f32, tag="out0")
                v1 = pool.tile([128, L], f32, tag="v1")
                out1 = pool.tile([128, L], f32, tag="out1")
                for c in range(CH):
                    cs = slice(c * W, (c + 1) * W)
                    nc.vector.tensor_tensor(
                        v1[:, cs], xb[:, cs], efb[:, cs], Alu.subtract
                    )
                # H scan emits out0 = H + K1 directly; chunked + chained
                bounds = [0, L // 2, 3 * L // 4, L]  # shrinking tail chunks
                for c in range(len(bounds) - 1):
                    cs = slice(bounds[c], bounds[c + 1])
                    nc.vector.tensor_tensor_scan(
                        out0[:, cs], rfb[:, cs], rbb[:, cs],
                        seed[:, 0:1] if c == 0 else out0[:, cs.start - 1 : cs.start],
                        Alu.add, Alu.subtract,
                    )
                    # out1 = (out0 + b0) + v1
                    nc.vector.scalar_tensor_tensor(
                        out1[:, cs], out0[:, cs], float(b0), v1[:, cs],
                        Alu.add, Alu.add,
                    )
                    nc.sync.dma_start(out=out_d[rows, 0, cs], in_=out0[:, cs])
                    nc.sync.dma_start(out=out_d[rows, 1, cs], in_=out1[:, cs])

    nc.compile()
    return nc


def _get_program(j, b0, b1, B_c, L):
    key = (float(j), float(b0), float(b1), B_c, L)
    if key not in _PROGRAM_CACHE:
        _PROGRAM_CACHE[key] = _build_program(j, b0, b1, B_c, L)
    return _PROGRAM_CACHE[key]


def _reference_np(j, b, observations):
    """Literal numpy fallback for parameter regimes the fast path can't handle."""
    j = np.float32(np.asarray(j).reshape(-1)[0])
    b = np.asarray(b, np.float32)
    obs = np.asarray(observations)
    B, L = obs.shape
    values = np.array([-0.5, 0.5], np.float32)
    psi = j * values[:, None] * values[None, :]
    phi = b[obs][..., None] * values

    def step(msg, phi_i):
        tmp = phi_i[:, :, None] + psi[None, :, :] + msg[:, :, None]
        return tmp.max(axis=1).astype(np.float32)

    fwd = np.zeros((B, L, 2), np.float32)
    msg = np.zeros((B, 2), np.float32)
    for i in range(L - 1):
        msg = step(msg, phi[:, i])
        fwd[:, i + 1] = msg
    bwd = np.zeros((B, L, 2), np.float32)
    msg = np.zeros((B, 2), np.float32)
    for i in range(L - 1, 0, -1):
        msg = step(msg, phi[:, i])
        bwd[:, i - 1] = msg
    return np.ascontiguousarray(
        (phi + fwd + bwd).transpose(0, 2, 1).astype(np.float32)
    )


TRACE = False
LAST_RESULTS = None


def kernel(j, b, observations):
    from concourse.bass_utils import run_bass_kernel_spmd

    j_np = np.asarray(j, np.float32).reshape(-1)
    b_np = np.asarray(b, np.float32).reshape(-1)
    obs = np.asarray(observations)
    jf, b0, b1 = float(j_np[0]), float(b_np[0]), float(b_np[1])

    fast = (
        jf > 0
        and all(t >= 0 or t <= -jf for t in (b0, b1))
        and abs(b0 - jf) > 1e-8 * max(1.0, jf)  # K1 derivation needs sA != 0
    )
    if not fast:
        return _reference_np(j, b, observations)

    B, L = obs.shape
    B_c = B // N_CORES
    nc = _get_program(jf, b0, b1, B_c, L)

    obs8 = obs.astype(np.int8)
    CH = 4
    W = L // CH
    n_tiles = B_c // 128
    # block-pack each core shard: [n_tiles*CH, 128, W], block (t,c) = rows
    # [t*128,(t+1)*128) x cols [c*W,(c+1)*W)  (layout transform only)
    def pack(shard):
        blk = shard.reshape(n_tiles, 128, CH, W).transpose(0, 2, 1, 3)
        return np.ascontiguousarray(blk.reshape(n_tiles * CH, 128, W))

    in_maps = [
        {"obs": pack(obs8[c * B_c : (c + 1) * B_c])} for c in range(N_CORES)
    ]
    res = run_bass_kernel_spmd(
        nc, in_maps, core_ids=list(range(N_CORES)), trace=TRACE
    )
    global LAST_RESULTS
    LAST_RESULTS = res
    return np.concatenate([r["out"] for r in res.results], axis=0)


# revision 17
# speedup vs baseline: 1.0185x; 1.0185x over previous
"""Trainium2 Bass kernel for the max-plus (tropical) forward-backward chain.

Problem: 2-state max-plus message passing over length-4096 chains, batch 2048.
    psi = j * outer([-.5,.5],[-.5,.5]);  phi[b,i,s] = b[obs[b,i]] * values[s]
    forward/backward max-plus message scans + belief assembly.

Algorithm (exact reduction; valid for j > 0 and each b_s either >= 0 or
<= -j, which holds for the staged inputs):
  The message difference d = msg[1]-msg[0] follows a clamped walk
  d' = clamp(d + tau_i, -j/2, +j/2), tau_i = b[obs_i].  With e = -d this is
  e' = max(A(obs), e) + B(obs): one hardware tensor_tensor_scan(max, add)
  per direction.  The message level msg[0] accumulates
  v_i = (a - c_i) + rho_i,  rho_i = relu(-e_i - (j/2 - tau_i)).
  Forward prefix Rf + backward suffix Rb combine into one more scan:
  H_i = (rho_f_{i-1} + H_{i-1}) - rho_b_i, seeded with sum(rho_b) + K1 so the
  scan emits out0 = H + K1 directly.  Then
    out1 = (out0 + b0) + v1,   v1 = beta*o - ef_sh - eb_rev  (beta = b1-b0)
  using per-row scalar K1 = 0.5*sum(j/2 - tau) - j/4 derived from the
  accumulated sum of the A tile.

Sharding: pure data parallel - batch 2048 -> 8 cores x 256 sequences
(2 partition-tiles of 128); the L-scans stay on-device per core.

Engine split per [128, 4096] tile:
  DVE    : 3 scans (ef, eb, H/out0) + 2 stt (xf, xb) + v1 TT + out1 stt
  ScalarE: A build (int8->f32 affine, accum for K1), 2 relus
  GpSimd : B build (int8->f32 affine), pad-column memsets (note:
           gpsimd (add,max) op pair measured pathologically slow - avoid)
  sync   : DMAs

Measured (8 cores, axon trn2): HW exec ~113-117 us per NEFF execution,
rel-l2 error vs reference 2.9e-06.  DVE busy ~96 us of that (3 scans at
2 cyc/elem + 4 pointwise ops at 1 cyc/elem over 2 x [128, 4096] tiles);
~24 us is fixed preamble/drain + startup DMA latency.
"""

import sys

import numpy as np

if "/opt/trn_rl_repo" not in sys.path:
    sys.path.insert(0, "/opt/trn_rl_repo")

B_FULL, L_FULL, N_CORES = 2048, 4096, 8

_PROGRAM_CACHE = {}


def _build_program(j, b0, b1, B_c, L):
    """Build + compile the per-core program: obs int8 [B_c, L] -> out f32 [B_c,2,L]."""
    import concourse.bacc as bacc
    import concourse.mybir as mybir
    import concourse.tile as tile

    f32 = mybir.dt.float32
    Alu = mybir.AluOpType
    Act = mybir.ActivationFunctionType

    a = j / 4.0
    twoa = j / 2.0
    beta = b1 - b0
    taus = [b0, b1]
    As, Bs = [], []
    for t in taus:
        if t >= 0:
            As.append(t - twoa)  # only the upper clamp can bind
            Bs.append(-t)
        else:  # t <= -j: guaranteed reset to the bottom clamp
            As.append(twoa)
            Bs.append(0.0)
    sA, bA = As[1] - As[0], As[0]
    sB, bB = Bs[1] - Bs[0], Bs[0]
    assert abs(sA) > 1e-8  # caller guarantees (K1 derives from sum(A))
    # K1 = 0.5*sum(Omega) - a, Omega = twoa - tau:
    #   sumOm = twoa*L - (b0*L + beta*N1), N1 = (sumA - bA*L)/sA
    # => K1 = cK0 + cK1 * sumA
    cK1 = -0.5 * beta / sA
    cK0 = 0.5 * (twoa * L - b0 * L + beta * bA * L / sA) - a
    bR = -(twoa - b0)  # relu bias: rho = relu(xf + bR), xf = beta*o - e_sh

    n_tiles = B_c // 128
    assert B_c % 128 == 0

    nc = bacc.Bacc("TRN2", target_bir_lowering=False, debug=False)
    CH = 4
    W = L // CH
    # obs arrives block-packed (host layout transform): block (t, c) holds
    # rows [t*128,(t+1)*128) x cols [c*W,(c+1)*W) contiguously, so every
    # chunk DMA is a single contiguous 128KB burst.
    obs_d = nc.dram_tensor(
        "obs", [(B_c // 128) * CH, 128, W], mybir.dt.int8, kind="ExternalInput"
    ).ap()
    out_d = nc.dram_tensor("out", [B_c, 2, L], f32, kind="ExternalOutput").ap()

    with tile.TileContext(nc) as tc:
        with tc.tile_pool(name="const", bufs=1) as cpool, tc.tile_pool(
            name="work", bufs=1
        ) as pool, tc.tile_pool(name="inp", bufs=2) as ipool:
            bA_t = cpool.tile([128, 1], f32, tag="bA")
            bR_t = cpool.tile([128, 1], f32, tag="bR")
            nc.vector.memset(bA_t[:], bA)
            nc.vector.memset(bR_t[:], bR)

            for t in range(n_tiles):
                rows = slice(t * 128, (t + 1) * 128)
                o = ipool.tile([128, L], mybir.dt.int8, tag="o")
                A = pool.tile([128, L], f32, tag="A")
                B = pool.tile([128, L], f32, tag="B")
                sA_p = [pool.tile([128, 1], f32, name=f"sAp{t}_{c}", tag=f"sA{c}") for c in range(CH + 1)]
                efb = pool.tile([128, L + 1], f32, tag="efb")
                ebb = pool.tile([128, L + 1], f32, tag="ebb")
                nc.gpsimd.memset(efb[:, 0:1], 0.0)
                nc.gpsimd.memset(ebb[:, 0:1], 0.0)

                for c in range(CH):
                    cs = slice(c * W, (c + 1) * W)
                    nc.sync.dma_start(out=o[:, cs], in_=obs_d[t * CH + c, :, :])
                # A/B builds and the forward walk use a tiny first chunk so the
                # first scan starts as soon as the first DMA lands
                ebounds = [0, 256, W, 2 * W, 3 * W, L] if t == 0 else [
                    0, W, 2 * W, 3 * W, L
                ]
                for c in range(len(ebounds) - 1):
                    cs = slice(ebounds[c], ebounds[c + 1])
                    nc.scalar.activation(
                        A[:, cs], o[:, cs], Act.Identity, bias=bA_t[:], scale=sA,
                        accum_out=sA_p[c][:],
                    )
                    nc.gpsimd.tensor_scalar(
                        B[:, cs], o[:, cs], sB, bB, Alu.mult, Alu.add
                    )
                    nc.vector.tensor_tensor_scan(
                        efb[:, 1 + ebounds[c] : 1 + ebounds[c + 1]],
                        A[:, cs], B[:, cs],
                        0.0 if c == 0 else efb[:, ebounds[c] : ebounds[c] + 1],
                        Alu.max, Alu.add,
                    )
                # backward walk (single full-length scan on reversed inputs;
                # full-length reversed reads carry no chunk penalty)
                nc.vector.tensor_tensor_scan(
                    ebb[:, 1 : L + 1], A[:, ::-1], B[:, ::-1], 0.0,
                    Alu.max, Alu.add,
                )

                xf = pool.tile([128, L], f32, tag="xf")
                rfb = pool.tile([128, L + 1], f32, tag="rfb")
                xb = pool.tile([128, L], f32, tag="xb")
                rbb = pool.tile([128, L], f32, tag="rbb")
                rb_p = [pool.tile([128, 1], f32, name=f"rbp{t}_{c}", tag=f"rb{c}") for c in range(CH)]
                nc.gpsimd.memset(rfb[:, 0:1], 0.0)
                for c in range(CH):
                    cs = slice(c * W, (c + 1) * W)
                    # xf = beta*o - ef_sh ; rho_f = relu(xf + bR)
                    nc.vector.scalar_tensor_tensor(
                        xf[:, cs], o[:, cs], beta, efb[:, cs], Alu.mult, Alu.subtract
                    )
                    nc.scalar.activation(
                        rfb[:, 1 + c * W : 1 + (c + 1) * W], xf[:, cs], Act.Relu,
                        bias=bR_t[:],
                    )
                    # xb in FORWARD layout: xb_k = beta*o_k - eb_fwd_k, where
                    # eb_fwd_k = ebb[:, L-1-k] (reversed read of the bwd walk)
                    nc.vector.scalar_tensor_tensor(
                        xb[:, cs], o[:, cs], beta,
                        ebb[:, L - (c + 1) * W : L - c * W][:, ::-1],
                        Alu.mult, Alu.subtract,
                    )
                    nc.scalar.activation(
                        rbb[:, cs], xb[:, cs], Act.Relu, bias=bR_t[:],
                        accum_out=rb_p[c][:],
                    )

                # seed = K1 + sum(rho_b),  K1 = cK0 + cK1*sumA  (tiny [128,1]
                # ops on gpsimd, off the DVE critical path)
                sA_t = pool.tile([128, 1], f32, tag="sAt")
                n_ab = len(ebounds) - 1
                nc.gpsimd.tensor_tensor(sA_t[:], sA_p[0][:], sA_p[1][:], Alu.add)
                for c in range(2, n_ab):
                    nc.gpsimd.tensor_tensor(sA_t[:], sA_t[:], sA_p[c][:], Alu.add)
                seed = pool.tile([128, 1], f32, tag="seed")
                nc.gpsimd.tensor_scalar(seed[:], sA_t[:], cK1, cK0, Alu.mult, Alu.add)
                for c in range(CH):
                    nc.gpsimd.tensor_tensor(seed[:], seed[:], rb_p[c][:], Alu.add)

                # v1 = xb_fwd - ef_sh (independent of H; emitted first so the
                # DVE can fill the rb-accum wait)
                out0 = pool.tile([128, L], f32, tag="out0")
                v1 = pool.tile([128, L], f32, tag="v1")
                out1 = pool.tile([128, L], f32, tag="out1")
                for c in range(CH):
                    cs = slice(c * W, (c + 1) * W)
                    nc.vector.tensor_tensor(
                        v1[:, cs], xb[:, cs], efb[:, cs], Alu.subtract
                    )
                # H scan emits out0 = H + K1 directly; chunked + chained
                bounds = [0, L // 2, 3 * L // 4, L]  # shrinking tail chunks
                for c in range(len(bounds) - 1):
                    cs = slice(bounds[c], bounds[c + 1])
                    nc.vector.tensor_tensor_scan(
                        out0[:, cs], rfb[:, cs], rbb[:, cs],
                        seed[:, 0:1] if c == 0 else out0[:, cs.start - 1 : cs.start],
                        Alu.add, Alu.subtract,
                    )
                    # out1 = (out0 + b0) + v1
                    nc.vector.scalar_tensor_tensor(
                        out1[:, cs], out0[:, cs], float(b0), v1[:, cs],
                        Alu.add, Alu.add,
                    )
                    nc.sync.dma_start(out=out_d[rows, 0, cs], in_=out0[:, cs])
                    nc.sync.dma_start(out=out_d[rows, 1, cs], in_=out1[:, cs])

    nc.compile()
    return nc


def _get_program(j, b0, b1, B_c, L):
    key = (float(j), float(b0), float(b1), B_c, L)
    if key not in _PROGRAM_CACHE:
        _PROGRAM_CACHE[key] = _build_program(j, b0, b1, B_c, L)
    return _PROGRAM_CACHE[key]


def _reference_np(j, b, observations):
    """Literal numpy fallback for parameter regimes the fast path can't handle."""
    j = np.float32(np.asarray(j).reshape(-1)[0])
    b = np.asarray(b, np.float32)
    obs = np.asarray(observations)
    B, L = obs.shape
    values = np.array([-0.5, 0.5], np.float32)
    psi = j * values[:, None] * values[None, :]
    phi = b[obs][..., None] * values

    def step(msg, phi_i):
        tmp = phi_i[:, :, None] + psi[None, :, :] + msg[:, :, None]
        return tmp.max(axis=1).astype(np.float32)

    fwd = np.zeros((B, L, 2), np.float32)
    msg = np.zeros((B, 2), np.float32)
    for i in range(L - 1):
        msg = step(msg, phi[:, i])
        fwd[:, i + 1] = msg
    bwd = np.zeros((B, L, 2), np.float32)
    msg = np.zeros((B, 2), np.float32)
    for i in range(L - 1, 0, -1):
        msg = step(msg, phi[:, i])
        bwd[:, i - 1] = msg
    return np.ascontiguousarray(
        (phi + fwd + bwd).transpose(0, 2, 1).astype(np.float32)
    )


TRACE = False
LAST_RESULTS = None


def kernel(j, b, observations):
    from concourse.bass_utils import run_bass_kernel_spmd

    j_np = np.asarray(j, np.float32).reshape(-1)
    b_np = np.asarray(b, np.float32).reshape(-1)
    obs = np.asarray(observations)
    jf, b0, b1 = float(j_np[0]), float(b_np[0]), float(b_np[1])

    fast = (
        jf > 0
        and all(t >= 0 or t <= -jf for t in (b0, b1))
        and abs(b0 - jf) > 1e-8 * max(1.0, jf)  # K1 derivation needs sA != 0
    )
    if not fast:
        return _reference_np(j, b, observations)

    B, L = obs.shape
    B_c = B // N_CORES
    nc = _get_program(jf, b0, b1, B_c, L)

    obs8 = obs.astype(np.int8)
    CH = 4
    W = L // CH
    n_tiles = B_c // 128
    # block-pack each core shard: [n_tiles*CH, 128, W], block (t,c) = rows
    # [t*128,(t+1)*128) x cols [c*W,(c+1)*W)  (layout transform only)
    def pack(shard):
        blk = shard.reshape(n_tiles, 128, CH, W).transpose(0, 2, 1, 3)
        return np.ascontiguousarray(blk.reshape(n_tiles * CH, 128, W))

    in_maps = [
        {"obs": pack(obs8[c * B_c : (c + 1) * B_c])} for c in range(N_CORES)
    ]
    res = run_bass_kernel_spmd(
        nc, in_maps, core_ids=list(range(N_CORES)), trace=TRACE
    )
    global LAST_RESULTS
    LAST_RESULTS = res
    return np.concatenate([r["out"] for r in res.results], axis=0)
